# revision 1
# baseline (speedup 1.0000x reference)
"""DgCD forward (topk channel masking) on 8 Trainium2 NeuronCores.

Bit-faithful replication of the XLA-CPU reference on the numerically critical
path (standardization -> gram -> minmax ratios -> scores -> top-k boundary),
sharded along batch (16 rows/core).  The mask decision runs in log-space
(g = ln(r) * inv_scores); its ordering matches the reference's pow-space
ordering at the boundary (validated: min boundary gap ~7e-6 rel >> our error).
"""
import os
import sys
sys.path.insert(0, "/opt/trn_rl_repo")
import numpy as np
from contextlib import ExitStack

import concourse.bass as bass
import concourse.bacc as bacc_mod
import concourse.mybir as mybir
import concourse.tile as tile
from concourse.bass_utils import run_bass_kernel_spmd

f32 = mybir.dt.float32
i32 = mybir.dt.int32
u8 = mybir.dt.uint8
AL = mybir.AluOpType
AF = mybir.ActivationFunctionType
AX = mybir.AxisListType

B, C, HW = 128, 2048, 196
NCORES = 8
BL = B // NCORES          # 16 batch rows per core
NT = C // 128             # 16 channel tiles (transposed layout)
NQ = C // 256             # 8 channel chunks (packed tail layout)
SEARCH_ROUNDS = 26
LO0, HI0 = -104.0, 0.0

C196 = float(np.float32(1.0 / 196.0))    # XLA's fp32(1/196)
C31 = float(np.float32(1.0 / 31.0))
C127 = float(np.float32(1.0 / 127.0))
LN2_HI = float(np.float32(0.693145751953125))
LN2_LO = float(np.float32(1.4286068e-06))
SQRT2 = float(np.float32(np.sqrt(2.0)))

_CACHE = {}


def _consts():
    ident = np.eye(128, dtype=np.float32)
    E64 = np.zeros((64, C), np.float32)      # stats (i,e) -> wide (i,b)
    for i in range(NT):
        for e in range(4):
            E64[i * 4 + e, i * 128 + e * 32:i * 128 + (e + 1) * 32] = 1.0
    E16w = np.zeros((16, C), np.float32)     # stats (i) -> wide (i,b)
    for i in range(NT):
        E16w[i, i * 128:(i + 1) * 128] = 1.0
    E16c = np.zeros((128, 16), np.float32)   # count combine  [128,16]
    for p in range(128):
        E16c[p, p // 8] = 1.0
    E16b = E16c.T.copy()                     # row -> partitions bcast [16,128]
    Eh0 = np.zeros((16, 128), np.float32)    # gram chunk halves
    Eh1 = np.zeros((16, 128), np.float32)
    for p in range(128):
        Eh0[2 * (p % 8), p] = 1.0
        Eh1[2 * (p % 8) + 1, p] = 1.0
    ones1 = np.ones((1, 128), np.float32)
    return {"ident": ident, "E64": E64, "E16w": E16w, "E16c": E16c,
            "E16b": E16b, "Eh0": Eh0, "Eh1": Eh1, "ones1": ones1}


def build(k, rho):
    nc = bacc_mod.Bacc()
    x_d = nc.dram_tensor("x", [BL, C, HW], f32, kind="ExternalInput")
    r_d = nc.dram_tensor("r", [BL, C], f32, kind="ExternalInput")
    cd = {n: nc.dram_tensor(n, list(v.shape), f32, kind="ExternalInput")
          for n, v in _consts().items()}
    out_d = nc.dram_tensor("out", [BL, C, HW], f32, kind="ExternalOutput")

    RHO = float(np.float32(rho))
    KF = float(k)
    # mask.sum() == B*C - B*k exactly (count(g>thr)==k per row, no boundary
    # ties), matching the reference's f32 divide bitwise.
    SCALE = float(np.float32(float(B * C)) / np.float32(float(B * C - B * k)))

    with tile.TileContext(nc) as tc, ExitStack() as ctx:
        pool = ctx.enter_context(tc.tile_pool(name="main", bufs=1))
        big = ctx.enter_context(tc.tile_pool(name="bigp", bufs=1))
        psum = ctx.enter_context(tc.tile_pool(name="psum", bufs=1, space="PSUM"))

        _n = iter(range(100000))

        def psA():
            return psum.tile([128, 1024], f32, tag="psA", bufs=1,
                             name=f"psA_{next(_n)}")

        def psB(shape):
            return psum.tile(shape, f32, tag="psB", bufs=3,
                             name=f"psB_{next(_n)}", padded_shape=[128, 256])

        def psC(shape):
            return psum.tile(shape, f32, tag="psC", bufs=3,
                             name=f"psC_{next(_n)}", padded_shape=[128, 1])
        dram = ctx.enter_context(tc.tile_pool(name="dram", bufs=1, space="DRAM"))
        cpool = ctx.enter_context(tc.tile_pool(name="cio", bufs=7))

        def bigt(n, shape=None):
            return big.tile(shape or [B, C], f32, tag=f"big{n}",
                            name=f"big{n}_{next(_n)}")

        # ---- constants ----
        cs = {}
        for n, v in _consts().items():
            cs[n] = pool.tile(list(v.shape), f32, tag="c_" + n, name="c_" + n)
            nc.gpsimd.dma_start(cs[n][:], cd[n][:])
        ident = cs["ident"]

        def sbuf_copy(ps, tag, shape=None, bufs=1):
            t = pool.tile(shape or [ps.shape[0], ps.shape[1]], f32, tag=tag,
                          name=f"sc_{tag}_{next(_n)}", bufs=bufs)
            nc.scalar.copy(t[:], ps[:])
            return t

        # =========== PHASE A (natural packed layout, contiguous DMA) ===========
        CK = 8                       # channels per partition per chunk
        WCH = CK * HW                # 1568 floats / 6.1KB per partition
        xv = x_d.rearrange("b (q j) h -> (b q) (j h)", q=NQ)
        avg_pack = pool.tile([128, 256], f32, tag="avg_pack")
        for m in range(256 // CK):
            xt = cpool.tile([128, WCH], f32, tag="xc")
            nc.sync.dma_start(xt[:], xv[:, m * WCH:(m + 1) * WCH])
            nc.vector.reduce_sum(avg_pack[:, m * CK:(m + 1) * CK],
                                 xt.rearrange("p (c h) -> p c h", h=HW)[:],
                                 axis=AX.X)
        nc.vector.tensor_scalar(avg_pack[:], avg_pack[:], C196, None, AL.mult)
        # repack (b,q)x(j) -> transposed-wide [c%128, i*BL+b], i = 2q+h
        avgT_loc = pool.tile([128, NT * BL], f32, tag="avgT_loc")
        avd2 = avgT_loc.rearrange("c (q t b) -> c q t b", q=NQ, t=2)
        for h in range(2):
            tp = psB([128, 128])
            nc.tensor.transpose(tp[:], avg_pack[:, h * 128:(h + 1) * 128],
                                ident[:])
            nc.vector.tensor_copy(avd2[:, :, h, :],
                                  tp.rearrange("c (b q) -> c q b", q=NQ)[:])

        # =========== AllGather avg ===========
        ag_in = dram.tile([128, NT * BL], f32, tag="ag_in")
        ag_out = dram.tile([NCORES, 128, NT * BL], f32, tag="ag_out")
        nc.sync.dma_start(ag_in[:], avgT_loc[:])
        nc.gpsimd.collective_compute(
            "AllGather", AL.bypass, replica_groups=[list(range(NCORES))],
            ins=[ag_in.opt()], outs=[ag_out.opt()])
        avg_T = bigt(0, [128, NT, B])        # [chan, i, b_glob]
        agv = ag_out.rearrange("r c (i b) -> c i r b", i=NT)
        avd = avg_T.rearrange("c i (r b) -> c i r b", r=NCORES)
        for i in range(NT):
            nc.sync.dma_start(avd[:, i, :, :], agv[:, i, :, :])
        avgTw = avg_T.rearrange("c i b -> c (i b)")

        # =========== B1: stats (transposed wide) ===========
        esum = pool.tile([128, NT * 4], f32, tag="esum")
        nc.vector.reduce_sum(esum[:], avgTw.rearrange("c (q w) -> c q w", w=32)[:],
                             axis=AX.X)
        m_all = pool.tile([128, NT * 4], f32, tag="m_all")
        nc.vector.tensor_scalar(m_all[:], esum[:], 0.03125, None, AL.mult)
        tsum = pool.tile([128, NT], f32, tag="tsum")
        nc.vector.reduce_sum(tsum[:], esum.rearrange("c (i e) -> c i e", e=4)[:],
                             axis=AX.X)
        tm_all = pool.tile([128, NT], f32, tag="tm_all")
        nc.vector.tensor_scalar(tm_all[:], tsum[:], 0.0078125, None, AL.mult)

        def statT(src, tag):
            """[128, K] stats -> transposed SBUF [K, 128]"""
            tp = psB([src.shape[1], 128])
            nc.tensor.transpose(tp[:], src[:], ident[:])
            return sbuf_copy(tp, "sT_" + tag)

        def bcast_T(srcT, Emat, tag):
            """stats-T [K,128] x E [K, 2048] -> SBUF [128, 2048] broadcast"""
            out = pool.tile([128, C], f32, tag=tag, name=f"bc_{tag}_{next(_n)}")
            for h in range(2):
                ps = psA()
                for ch in range(2):
                    col = h * 1024 + ch * 512
                    nc.tensor.matmul(ps[:, ch * 512:(ch + 1) * 512], srcT[:],
                                     Emat[:, col:col + 512],
                                     start=True, stop=True)
                nc.scalar.copy(out[:, h * 1024:(h + 1) * 1024], ps[:])
            return out

        mT = statT(m_all, "m")
        Menv = bcast_T(mT, cs["E64"], "bcR")
        cen_e = bigt(1)
        nc.vector.tensor_sub(cen_e[:], avgTw[:], Menv[:])
        tmT = statT(tm_all, "tm")
        Mtot = bcast_T(tmT, cs["E16w"], "bcR")
        cen_t = bigt(2)
        nc.vector.tensor_sub(cen_t[:], avgTw[:], Mtot[:])

        sq = bigt(3)
        nc.vector.tensor_mul(sq[:], cen_e[:], cen_e[:])
        vsum_e = pool.tile([128, NT * 4], f32, tag="vsum_e")
        nc.vector.reduce_sum(vsum_e[:], sq.rearrange("c (q w) -> c q w", w=32)[:],
                             axis=AX.X)
        nc.vector.tensor_mul(sq[:], cen_t[:], cen_t[:])
        vwin = pool.tile([128, NT * 4], f32, tag="vwin")
        nc.vector.reduce_sum(vwin[:], sq.rearrange("c (q w) -> c q w", w=32)[:],
                             axis=AX.X)
        vsum_t = pool.tile([128, NT], f32, tag="vsum_t")
        nc.vector.reduce_sum(vsum_t[:], vwin.rearrange("c (i e) -> c i e", e=4)[:],
                             axis=AX.X)

        def _cr_sqrt(out, a, tag):
            shape = list(out.shape)
            def st(nm):
                return pool.tile(shape, f32, tag=tag + nm, name=tag + nm)
            y0 = st("_y0")
            nc.scalar.activation(y0[:], a[:], AF.Sqrt)
            ry = st("_ry")
            nc.vector.reciprocal(ry[:], y0[:])
            t = st("_t")
            nc.vector.tensor_mul(t[:], a[:], ry[:])
            y1 = st("_y1")
            nc.vector.tensor_add(y1[:], y0[:], t[:])
            nc.vector.tensor_scalar(y1[:], y1[:], 0.5, None, AL.mult)
            c = st("_c")
            nc.vector.tensor_scalar(c[:], y1[:], 4097.0, None, AL.mult)
            hi = st("_hi")
            nc.vector.tensor_sub(hi[:], c[:], y1[:])
            nc.vector.tensor_sub(hi[:], c[:], hi[:])
            lo = st("_lo")
            nc.vector.tensor_sub(lo[:], y1[:], hi[:])
            p = st("_p")
            nc.vector.tensor_mul(p[:], y1[:], y1[:])
            e1 = st("_e1")
            nc.vector.tensor_mul(e1[:], hi[:], hi[:])
            nc.vector.tensor_sub(e1[:], e1[:], p[:])
            hl = st("_hl")
            nc.vector.tensor_mul(hl[:], hi[:], lo[:])
            nc.vector.tensor_scalar(hl[:], hl[:], 2.0, None, AL.mult)
            nc.vector.tensor_add(e1[:], e1[:], hl[:])
            nc.vector.tensor_mul(hl[:], lo[:], lo[:])
            nc.vector.tensor_add(e1[:], e1[:], hl[:])
            rem = st("_rm")
            nc.vector.tensor_sub(rem[:], a[:], p[:])
            nc.vector.tensor_sub(rem[:], rem[:], e1[:])
            nc.vector.reciprocal(ry[:], y1[:])
            nc.vector.tensor_mul(rem[:], rem[:], ry[:])
            nc.vector.tensor_scalar(rem[:], rem[:], 0.5, None, AL.mult)
            nc.vector.tensor_add(out[:], y1[:], rem[:])

        var_e = pool.tile([128, NT * 4], f32, tag="var_e")
        nc.vector.tensor_scalar(var_e[:], vsum_e[:], C31, None, AL.mult)
        nc.vector.tensor_scalar(var_e[:], var_e[:], 1e-05, None, AL.add)
        sd_e = pool.tile([128, NT * 4], f32, tag="sd_e")
        _cr_sqrt(sd_e, var_e, "cse")
        rsd_e = pool.tile([128, NT * 4], f32, tag="rsd_e")
        nc.vector.reciprocal(rsd_e[:], sd_e[:])
        var_t = pool.tile([128, NT], f32, tag="var_t")
        nc.vector.tensor_scalar(var_t[:], vsum_t[:], C127, None, AL.mult)
        nc.vector.tensor_scalar(var_t[:], var_t[:], 1e-05, None, AL.add)
        sd_t = pool.tile([128, NT], f32, tag="sd_t")
        _cr_sqrt(sd_t, var_t, "cst")
        rsd_t = pool.tile([128, NT], f32, tag="rsd_t")
        nc.vector.reciprocal(rsd_t[:], sd_t[:])

        def mark_tt(out, num, R, D, qtag, ttag):
            q0 = bigt(qtag)
            nc.vector.tensor_mul(q0[:], num[:], R[:])
            t = bigt(ttag)
            nc.vector.tensor_mul(t[:], q0[:], D[:])
            nc.vector.tensor_sub(t[:], num[:], t[:])
            nc.vector.tensor_mul(t[:], t[:], R[:])
            nc.vector.tensor_add(out[:], q0[:], t[:])

        DeS = bcast_T(statT(sd_e, "sde"), cs["E64"], "bcD")
        ReS = bcast_T(statT(rsd_e, "rse"), cs["E64"], "bcR")
        z_e = bigt(4)
        mark_tt(z_e, cen_e, ReS, DeS, 8, 9)
        DtS = bcast_T(statT(sd_t, "sdt"), cs["E16w"], "bcD")
        RtS = bcast_T(statT(rsd_t, "rst"), cs["E16w"], "bcR")
        z_t = bigt(5)
        mark_tt(z_t, cen_t, RtS, DtS, 8, 9)

        # transpose z -> row layout
        ze_row = bigt(6)
        zt_row = bigt(7)
        zv_e = z_e.rearrange("c (i b) -> c i b", i=NT)
        zv_t = z_t.rearrange("c (i b) -> c i b", i=NT)
        for i in range(NT):
            tp = psB([128, 128])
            nc.tensor.transpose(tp[:], zv_e[:, i, :], ident[:])
            nc.scalar.copy(ze_row[:, i * 128:(i + 1) * 128], tp[:])
            tp2 = psB([128, 128])
            nc.tensor.transpose(tp2[:], zv_t[:, i, :], ident[:])
            nc.scalar.copy(zt_row[:, i * 128:(i + 1) * 128], tp2[:])

        # =========== B2: robust middle (row layout) ===========
        def softmax_parts(z_row, shtag, extag, lsftag, prtag, sfx):
            mx = pool.tile([B, 1], f32, tag="mx" + sfx)
            nc.vector.tensor_reduce(mx[:], z_row[:], axis=AX.X, op=AL.max)
            sh = bigt(shtag)
            nc.vector.tensor_scalar(sh[:], z_row[:], mx[:], None, AL.subtract)
            es = pool.tile([B, 1], f32, tag="es" + sfx)
            ex = bigt(extag)
            nc.scalar.activation(ex[:], sh[:], AF.Exp)
            nc.vector.reduce_sum(es[:], ex.rearrange("b (o c) -> b o c", o=1)[:],
                                 axis=AX.X)
            ls = pool.tile([B, 1], f32, tag="ls" + sfx)
            nc.scalar.activation(ls[:], es[:], AF.Ln)
            ng = pool.tile([B, 1], f32, tag="ng" + sfx)
            nc.vector.tensor_scalar(ng[:], ls[:], -1.0, None, AL.mult)
            nc.scalar.activation(ng[:], ng[:], AF.Exp)
            nc.vector.tensor_mul(ng[:], es[:], ng[:])
            nc.vector.tensor_scalar(ng[:], ng[:], 1.0, None, AL.subtract)
            nc.vector.tensor_add(ls[:], ls[:], ng[:])
            lsf = bigt(lsftag)
            nc.vector.tensor_scalar(lsf[:], sh[:], ls[:], None, AL.subtract)
            pr = bigt(prtag)
            nc.scalar.activation(pr[:], lsf[:], AF.Exp)
            return lsf, pr

        els, p_sm = softmax_parts(ze_row, 0, 1, 2, 8, "e")   # avg_T,cen_e dead
        tls, q_sm = softmax_parts(zt_row, 0, 1, 9, 5, "t")

        diff = bigt(0)
        nc.vector.tensor_sub(diff[:], els[:], tls[:])
        KL = pool.tile([B, 1], f32, tag="KL")
        pd = bigt(1)
        nc.vector.tensor_mul(pd[:], p_sm[:], diff[:])
        nc.vector.reduce_sum(KL[:], pd.rearrange("b (o c) -> b o c", o=1)[:],
                             axis=AX.X)
        G_env = bigt(3)
        nc.vector.tensor_scalar(G_env[:], diff[:], KL[:], None, AL.subtract)
        nc.vector.tensor_mul(G_env[:], p_sm[:], G_env[:])
        nc.vector.tensor_scalar(G_env[:], G_env[:], 0.0078125, None, AL.mult)
        G_tot = bigt(2)
        nc.vector.tensor_sub(G_tot[:], q_sm[:], p_sm[:])
        nc.vector.tensor_scalar(G_tot[:], G_tot[:], 0.0078125, None, AL.mult)
        g_ve = bigt(9)
        nc.vector.tensor_mul(g_ve[:], G_env[:], ze_row[:])
        g_vt = bigt(8)
        nc.vector.tensor_mul(g_vt[:], G_tot[:], zt_row[:])

        def pert_scale(g, sfx):
            n2 = pool.tile([B, 1], f32, tag="n2" + sfx)
            jk = bigt(1)
            nc.vector.tensor_mul(jk[:], g[:], g[:])
            nc.vector.reduce_sum(n2[:], jk.rearrange("b (o c) -> b o c", o=1)[:],
                                 axis=AX.X)
            nc.scalar.activation(n2[:], n2[:], AF.Sqrt)
            nc.vector.tensor_scalar(n2[:], n2[:], 1e-12, None, AL.add)
            nc.scalar.activation(n2[:], n2[:], AF.Sqrt)
            nc.vector.reciprocal(n2[:], n2[:])
            nc.vector.tensor_scalar(n2[:], n2[:], RHO, None, AL.mult)
            return n2

        s_me = pert_scale(G_env, "a")
        s_ve = pert_scale(g_ve, "b")
        s_mt = pert_scale(G_tot, "c")
        s_vt = pert_scale(g_vt, "d")
        d_me = bigt(4)
        nc.vector.tensor_scalar(d_me[:], G_env[:], s_me[:], None, AL.mult)
        d_ve = bigt(3)   # overwrites G_env (dead)
        nc.vector.tensor_scalar(d_ve[:], g_ve[:], s_ve[:], None, AL.mult)
        d_mt = bigt(9)   # g_ve dead
        nc.vector.tensor_scalar(d_mt[:], G_tot[:], s_mt[:], None, AL.mult)
        d_vt = bigt(2)   # G_tot dead
        nc.vector.tensor_scalar(d_vt[:], g_vt[:], s_vt[:], None, AL.mult)

        env_a2 = bigt(0)
        nc.vector.tensor_scalar(d_ve[:], d_ve[:], 1.0, None, AL.add)
        nc.vector.tensor_mul(env_a2[:], ze_row[:], d_ve[:])
        nc.vector.tensor_add(env_a2[:], env_a2[:], d_me[:])
        tot_a2 = bigt(1)
        nc.vector.tensor_scalar(d_vt[:], d_vt[:], 1.0, None, AL.add)
        nc.vector.tensor_mul(tot_a2[:], zt_row[:], d_vt[:])
        nc.vector.tensor_add(tot_a2[:], tot_a2[:], d_mt[:])

        # a2 -> DRAM scratch for per-core packed readback
        a2s_t = dram.tile([B, C], f32, tag="a2s_t")
        a2s_e = dram.tile([B, C], f32, tag="a2s_e")
        nc.sync.dma_start(a2s_t[:], tot_a2[:])
        nc.sync.dma_start(a2s_e[:], env_a2[:])

        # w = (tot+1e-7)*(env+1e-7); transpose; gram sums
        wrow = bigt(6)   # ze_row dead
        nc.vector.tensor_scalar(wrow[:], tot_a2[:], 1e-07, None, AL.add)
        w2 = bigt(7)     # zt_row dead
        nc.vector.tensor_scalar(w2[:], env_a2[:], 1e-07, None, AL.add)
        nc.vector.tensor_mul(wrow[:], wrow[:], w2[:])
        gram_T = pool.tile([128, NT], f32, tag="gram_T")
        for i in range(NT):
            wtp = psB([128, 128])
            nc.tensor.transpose(wtp[:], wrow[:, i * 128:(i + 1) * 128], ident[:])
            gw = pool.tile([128, 4], f32, tag="gw")
            nc.vector.reduce_sum(gw[:], wtp.rearrange("c (e w) -> c e w", e=4)[:],
                                 axis=AX.X)
            nc.vector.reduce_sum(gram_T[:, i:i + 1],
                                 gw.rearrange("c (o e) -> c o e", o=1)[:], axis=AX.X)
        rgram_T = pool.tile([128, NT], f32, tag="rgram_T")
        nc.vector.reciprocal(rgram_T[:], gram_T[:])

        # =========== TAIL (sharded, packed [128 = (b_loc,q), 256]) ===========
        pid = nc.gpsimd.partition_id()
        ta2 = pool.tile([128, 256], f32, tag="ta2")
        ea2 = pool.tile([128, 256], f32, tag="ea2")
        a2rt = a2s_t.rearrange("(r b) (q j) -> r (b q) j", b=BL, q=NQ)
        a2re = a2s_e.rearrange("(r b) (q j) -> r (b q) j", b=BL, q=NQ)
        nc.gpsimd.dma_start(ta2[:], a2rt[bass.ds(pid, 1), :, :])
        nc.gpsimd.dma_start(ea2[:], a2re[bass.ds(pid, 1), :, :])

        # gram / rgram packed broadcasts (PSUM)
        gT = statT(gram_T, "gT")     # [16, 128]
        rgT = statT(rgram_T, "rgT")
        Dg = psB([128, 256])
        nc.tensor.matmul(Dg[:, 0:128], cs["Eh0"][:], gT[:], start=True, stop=True)
        nc.tensor.matmul(Dg[:, 128:256], cs["Eh1"][:], gT[:], start=True, stop=True)
        Rg = psB([128, 256])
        nc.tensor.matmul(Rg[:, 0:128], cs["Eh0"][:], rgT[:], start=True, stop=True)
        nc.tensor.matmul(Rg[:, 128:256], cs["Eh1"][:], rgT[:], start=True, stop=True)
        DgS = sbuf_copy(Dg, "DgS")
        RgS = sbuf_copy(Rg, "RgS")

        def mark_p(out, num, R, D, tag):
            q0 = pool.tile([128, 256], f32, tag=tag + "q")
            nc.vector.tensor_mul(q0[:], num[:], R[:])
            t = pool.tile([128, 256], f32, tag=tag + "t")
            nc.vector.tensor_mul(t[:], q0[:], D[:])
            nc.vector.tensor_sub(t[:], num[:], t[:])
            nc.vector.tensor_mul(t[:], t[:], R[:])
            nc.vector.tensor_add(out[:], q0[:], t[:])

        t3 = pool.tile([128, 256], f32, tag="t3")
        mark_p(t3, ta2, RgS, DgS, "mp")
        e3 = pool.tile([128, 256], f32, tag="e3")
        mark_p(e3, ea2, RgS, DgS, "mp")

        def row_combine(val_pp, op, sfx, want_bc=True):
            """per-partition [128,1] -> per-row [16,1] SBUF (+[128,1] PSUM bcast)"""
            tp = psB([1, 128])
            nc.tensor.transpose(tp[:], val_pp[:], ident[:])
            s1 = sbuf_copy(tp, "rcs")
            red = pool.tile([1, 16], f32, tag="rcr", name=f"rcr_{next(_n)}")
            nc.vector.tensor_reduce(red[:], s1.rearrange("o (b q) -> o b q", q=8)[:],
                                    axis=AX.X, op=op)
            tp2 = psB([16, 1])
            nc.tensor.transpose(tp2[:], red[:], ident[0:1, 0:1])
            c16 = sbuf_copy(tp2, "rc16", bufs=4)
            if not want_bc:
                return c16, None
            bc = psC([128, 1])
            nc.tensor.matmul(bc[:], cs["E16b"][:], c16[:], start=True, stop=True)
            return c16, bc

        def minmax_norm(v, sfx):
            """(v - rowmin) / (rowmax - rowmin), markstein w/ per-partition scalars"""
            mxp = pool.tile([128, 1], f32, tag="mxp", name=f"mxp_{next(_n)}")
            nc.vector.tensor_reduce(mxp[:], v[:], axis=AX.X, op=AL.max)
            mnp = pool.tile([128, 1], f32, tag="mnp", name=f"mnp_{next(_n)}")
            nc.vector.tensor_reduce(mnp[:], v[:], axis=AX.X, op=AL.min)
            mx16, _ = row_combine(mxp, AL.max, "a", want_bc=False)
            mn16, mnbc = row_combine(mnp, AL.min, "b")
            num = pool.tile([128, 256], f32, tag="num" + sfx)
            nc.vector.tensor_scalar(num[:], v[:], mnbc[:], None, AL.subtract)
            den16 = pool.tile([16, 1], f32, tag="den16", name=f"den_{next(_n)}")
            nc.vector.tensor_sub(den16[:], mx16[:], mn16[:])
            rden16 = pool.tile([16, 1], f32, tag="rden16", name=f"rden_{next(_n)}")
            nc.vector.reciprocal(rden16[:], den16[:])
            dbc = psC([128, 1])
            nc.tensor.matmul(dbc[:], cs["E16b"][:], den16[:], start=True, stop=True)
            rbc = psC([128, 1])
            nc.tensor.matmul(rbc[:], cs["E16b"][:], rden16[:], start=True, stop=True)
            q0 = pool.tile([128, 256], f32, tag="mmq", name=f"mmq_{next(_n)}")
            nc.vector.tensor_scalar(q0[:], num[:], rbc[:], None, AL.mult)
            t = pool.tile([128, 256], f32, tag="mmt", name=f"mmt_{next(_n)}")
            nc.vector.tensor_scalar(t[:], q0[:], dbc[:], None, AL.mult)
            nc.vector.tensor_sub(t[:], num[:], t[:])
            nc.vector.tensor_scalar(t[:], t[:], rbc[:], None, AL.mult)
            nc.vector.tensor_add(num[:], q0[:], t[:])
            return num

        t4 = minmax_norm(t3, "t")
        e4 = minmax_norm(e3, "e")
        sqd = pool.tile([128, 256], f32, tag="sqd")
        nc.vector.tensor_sub(sqd[:], t4[:], e4[:])
        nc.vector.tensor_mul(sqd[:], sqd[:], sqd[:])

        # inv_s = (max-min) * recip(sq - min)
        mxp2 = pool.tile([128, 1], f32, tag="mxp2")
        nc.vector.tensor_reduce(mxp2[:], sqd[:], axis=AX.X, op=AL.max)
        mnp2 = pool.tile([128, 1], f32, tag="mnp2")
        nc.vector.tensor_reduce(mnp2[:], sqd[:], axis=AX.X, op=AL.min)
        mx216, _ = row_combine(mxp2, AL.max, "s1")
        mn216, mnbc2 = row_combine(mnp2, AL.min, "s2")
        num16 = pool.tile([16, 1], f32, tag="num16")
        nc.vector.tensor_sub(num16[:], mx216[:], mn216[:])
        numbc = psC([128, 1])
        nc.tensor.matmul(numbc[:], cs["E16b"][:], num16[:], start=True, stop=True)
        den2 = pool.tile([128, 256], f32, tag="den2")
        nc.vector.tensor_scalar(den2[:], sqd[:], mnbc2[:], None, AL.subtract)
        nc.vector.reciprocal(den2[:], den2[:])
        inv_s = pool.tile([128, 256], f32, tag="inv_s")
        nc.vector.tensor_scalar(inv_s[:], den2[:], numbc[:], None, AL.mult)

        # ln(r) (packed) and g
        rp = pool.tile([128, 256], f32, tag="rp")
        nc.sync.dma_start(rp[:], r_d.rearrange("b (q j) -> (b q) j", q=NQ)[:])
        lnr = pool.tile([128, 256], f32, tag="lnr")
        bits = rp.bitcast(i32)
        e_i = pool.tile([128, 256], i32, tag="ln_ei")
        nc.vector.tensor_scalar(e_i[:], bits[:], 23, None, AL.logical_shift_right)
        nc.vector.tensor_scalar(e_i[:], e_i[:], 127, None, AL.subtract)
        m_i = pool.tile([128, 256], i32, tag="ln_mi")
        nc.vector.tensor_scalar(m_i[:], bits[:], 0x7FFFFF, None, AL.bitwise_and)
        nc.vector.tensor_scalar(m_i[:], m_i[:], 0x3F800000, None, AL.bitwise_or)
        mf = m_i.bitcast(f32)
        e_f = pool.tile([128, 256], f32, tag="ln_ef")
        nc.vector.tensor_copy(e_f[:], e_i[:])
        sel = pool.tile([128, 256], f32, tag="ln_sel")
        nc.vector.tensor_scalar(sel[:], mf[:], SQRT2, None, AL.is_ge)
        mh = pool.tile([128, 256], f32, tag="ln_mh")
        nc.vector.tensor_scalar(mh[:], mf[:], 0.5, None, AL.mult)
        nc.vector.tensor_mul(mh[:], mh[:], sel[:])
        mm = pool.tile([128, 256], f32, tag="ln_mm")
        nc.vector.tensor_sub(mm[:], mf[:], mh[:])
        nc.vector.tensor_add(e_f[:], e_f[:], sel[:])
        lnum = pool.tile([128, 256], f32, tag="ln_nm")
        nc.vector.tensor_scalar(lnum[:], mm[:], 1.0, None, AL.subtract)
        lden = pool.tile([128, 256], f32, tag="ln_dn")
        nc.vector.tensor_scalar(lden[:], mm[:], 1.0, None, AL.add)
        nc.vector.reciprocal(lden[:], lden[:])
        tq = pool.tile([128, 256], f32, tag="ln_t")
        nc.vector.tensor_mul(tq[:], lnum[:], lden[:])
        tq2 = pool.tile([128, 256], f32, tag="ln_t2")
        nc.vector.tensor_mul(tq2[:], tq[:], tq[:])
        acc = pool.tile([128, 256], f32, tag="ln_ac")
        nc.vector.tensor_scalar(acc[:], tq2[:], float(np.float32(2.0 / 11.0)),
                                float(np.float32(2.0 / 9.0)), AL.mult, op1=AL.add)
        nc.vector.tensor_mul(acc[:], acc[:], tq2[:])
        nc.vector.tensor_scalar(acc[:], acc[:], float(np.float32(2.0 / 7.0)), None, AL.add)
        nc.vector.tensor_mul(acc[:], acc[:], tq2[:])
        nc.vector.tensor_scalar(acc[:], acc[:], float(np.float32(2.0 / 5.0)), None, AL.add)
        nc.vector.tensor_mul(acc[:], acc[:], tq2[:])
        nc.vector.tensor_scalar(acc[:], acc[:], float(np.float32(2.0 / 3.0)), None, AL.add)
        nc.vector.tensor_mul(tq2[:], tq[:], tq2[:])
        nc.vector.tensor_mul(acc[:], tq2[:], acc[:])
        nc.vector.tensor_scalar(tq[:], tq[:], 2.0, None, AL.mult)
        nc.vector.tensor_add(acc[:], tq[:], acc[:])
        nc.vector.tensor_scalar(lnum[:], e_f[:], LN2_LO, None, AL.mult)
        nc.vector.tensor_add(acc[:], acc[:], lnum[:])
        nc.vector.tensor_scalar(lnum[:], e_f[:], LN2_HI, None, AL.mult)
        nc.vector.tensor_add(lnr[:], acc[:], lnum[:])

        g = pool.tile([128, 256], f32, tag="g")
        nc.vector.tensor_mul(g[:], lnr[:], inv_s[:])

        # binary search for (k+1)-th largest g per row
        lo = pool.tile([16, 1], f32, tag="s_lo", bufs=2)
        nc.gpsimd.memset(lo[:], LO0)
        hi = pool.tile([16, 1], f32, tag="s_hi", bufs=2)
        nc.gpsimd.memset(hi[:], HI0)
        cjunk = pool.tile([128, 256], f32, tag="cjunk")
        for it in range(SEARCH_ROUNDS):
            mid = pool.tile([16, 1], f32, tag="s_mid")
            nc.vector.tensor_add(mid[:], lo[:], hi[:])
            nc.vector.tensor_scalar(mid[:], mid[:], 0.5, None, AL.mult)
            midbc = psC([128, 1])
            nc.tensor.matmul(midbc[:], cs["E16b"][:], mid[:], start=True, stop=True)
            cnt = pool.tile([128, 1], f32, tag="s_cnt")
            nc.vector.tensor_scalar(cjunk[:], g[:], midbc[:], None, AL.is_gt,
                                    op1=AL.add, accum_out=cnt[:])
            c16ps = psC([16, 1])
            nc.tensor.matmul(c16ps[:], cs["E16c"][:], cnt[:], start=True, stop=True)
            flag = pool.tile([16, 1], f32, tag="s_flag")
            nc.vector.tensor_scalar(flag[:], c16ps[:], KF, None, AL.is_gt)
            # if count > k: lo = mid else hi = mid
            dlt = pool.tile([16, 1], f32, tag="s_dlt")
            nc.vector.tensor_sub(dlt[:], mid[:], lo[:])
            nc.vector.tensor_mul(dlt[:], dlt[:], flag[:])
            lo2 = pool.tile([16, 1], f32, tag="s_lo", name=f"slo_{next(_n)}", bufs=2)
            nc.vector.tensor_add(lo2[:], lo[:], dlt[:])
            dlt2 = pool.tile([16, 1], f32, tag="s_dlt2")
            nc.vector.tensor_sub(dlt2[:], hi[:], mid[:])
            nc.vector.tensor_mul(dlt2[:], dlt2[:], flag[:])
            hi2 = pool.tile([16, 1], f32, tag="s_hi", name=f"shi_{next(_n)}", bufs=2)
            nc.vector.tensor_add(hi2[:], mid[:], dlt2[:])
            lo, hi = lo2, hi2

        # thr = rowmax(g where g <= hi)
        hibc = psC([128, 1])
        nc.tensor.matmul(hibc[:], cs["E16b"][:], hi[:], start=True, stop=True)
        selm = pool.tile([128, 256], u8, tag="selm")
        nc.vector.tensor_scalar(selm[:], g[:], hibc[:], None, AL.is_le)
        gm = pool.tile([128, 256], f32, tag="gm")
        nc.gpsimd.memset(gm[:], -1.0e38)
        nc.vector.copy_predicated(gm[:], selm[:], g[:])
        gmx = pool.tile([128, 1], f32, tag="gmx")
        nc.vector.tensor_reduce(gmx[:], gm[:], axis=AX.X, op=AL.max)
        thr16, thrbc = row_combine(gmx, AL.max, "th")

        # final mask, folded with the global scale.  count(g > thr) == k
        # exactly per row (validated: no ties at the boundary), so
        # mask.sum() == B*C - B*k and the scale is a compile-time constant.
        mask_s = pool.tile([128, 256], f32, tag="mask_s")
        nc.vector.tensor_scalar(mask_s[:], g[:], thrbc[:], None, AL.is_le)
        nc.vector.tensor_scalar(mask_s[:], mask_s[:], SCALE, None, AL.mult)

        # =========== PHASE C (natural packed layout, contiguous DMA) ===========
        # loads on the gpsimd queue so they prefetch under the middle
        # instead of stalling behind mask-gated stores on sync.
        ov = out_d.rearrange("b (q j) h -> (b q) (j h)", q=NQ)
        for m in range(256 // CK):
            xt = cpool.tile([128, WCH], f32, tag="xc")
            nc.gpsimd.dma_start(xt[:], xv[:, m * WCH:(m + 1) * WCH])
            for jj in range(CK):
                col = m * CK + jj
                sl = xt[:, jj * HW:(jj + 1) * HW]
                nc.vector.tensor_scalar(sl, sl, mask_s[:, col:col + 1],
                                        None, AL.mult)
            nc.sync.dma_start(ov[:, m * WCH:(m + 1) * WCH], xt[:])

    nc.finalize()
    return nc


def kernel(x, r, ratio, rho):
    x = np.ascontiguousarray(np.asarray(x, dtype=np.float32))
    r = np.ascontiguousarray(np.asarray(r, dtype=np.float32))
    ratio_f = float(np.asarray(ratio))
    rho_f = float(np.asarray(rho))
    k = int(ratio_f * C)
    key = (k, np.float32(rho_f).tobytes())
    if key not in _CACHE:
        _CACHE[key] = build(k, rho_f)
    nc = _CACHE[key]

    consts = _consts()
    xr = x.reshape(B, C, HW)
    in_maps = []
    for c in range(NCORES):
        m = {"x": np.ascontiguousarray(xr[c * BL:(c + 1) * BL]),
             "r": np.ascontiguousarray(r[c * BL:(c + 1) * BL])}
        m.update(consts)
        in_maps.append(m)
    trace_dir = os.environ.get("KTRACE_DIR") or None
    res = run_bass_kernel_spmd(nc, in_maps, core_ids=list(range(NCORES)),
                               trace=bool(trace_dir), tmpdir=trace_dir)
    if res.exec_time_ns is not None:
        print(f"HW exec time: {res.exec_time_ns} ns")
    global LAST_RESULT
    LAST_RESULT = res
    out = np.concatenate([res.results[c]["out"].reshape(BL, C, HW)
                          for c in range(NCORES)], axis=0)
    return out.reshape(B, C, 14, 14)

